# revision 18
# baseline (speedup 1.0000x reference)
"""Bass/Trainium2 kernel for batched 3D FFT circular convolution.

Reference computes y = Re(IFFT3(FFT3(x) * FFT3(w))) with net scaling
circular_conv3d(x, w) / sqrt(N); x: (16, 32, 128, 128) f32, w: (32, 128, 128).

Strategy (data parallel over batch, 8 cores x 2 samples):
- Pack two real samples as one complex volume z = x0 + i*x1; y0 = Re, y1 = Im.
- All FFTs as DFT matmuls in bf16 (inputs pre-rounded to bf16 on host).
- Transposes are FUSED into the DFT matmuls: the DFT matrix is symmetric, so
  making the DATA the stationary operand computes data^T @ F = (F @ data)^T --
  the transform output lands transposed (next axis on partitions) for free.
  Full-array weight loads hide in the PE background buffer; per block each
  component needs 2 LDW + 4 matmuls of 128 cols.
- PSUM is [R-half | I-half] so every eviction is a contiguous copy; layout
  permutations ride in single-strided stationary APs of the next stage:
    S1 evicts to f = 1024*k2q + 32*d1 + j  (32-elem contiguous j-runs), so
    S2's stationary for block j is (m = 32*k2q + d1) at stride 32 -- the
    exact (k2q,d1) partition order the block-diagonal d1-DFT needs.
    S4 evicts contiguously to f = 32*k2 + d1, so S5's stationary for block
    d1 is k2 at stride 32.
- Host pre-transposes x,w to (d2,d1,d3) and un-transposes y so every DMA run
  is >= 1KB contiguous (otherwise DMA is descriptor-rate bound).
- W~ = FFT3(w)/(N*sqrt(N)) computed on-device per core, replicated (an
  AllGather-sharded variant measured worse: cross-core start skew ~20us).
  W-chain stage groups interleave one stage ahead of the z-chain to fill
  the z-chain's all-to-all stage-boundary bubbles with PE work.

Stage layouts (partition | free):
  L0   [d2 | d1,d3]                  f = d1*128 + d3
  S1   fused FFT d2  -> [d3 | 1024*k2q + 32*d1 + j]   (k2 = 4j+k2q)
  S2   fused FFT d3  -> [(k2q,d1) | j,k3]  f = j*128 + k3
  S3   BD FFT d1 (weight-stationary) -> [(k2q,k1) | j,k3]
  M    V = Z * W~   (DVE, bf16 2x mode)
  S4   fused BD IFFT d1 -> [k3 | 32*k2 + d1]
  S5   fused IFFT k3 -> [k2 | d1,d3]  f = d1*128 + d3
  S6   IFFT k2 (weight-stationary) -> [d2 | d1,d3] -> DMA out
"""

import numpy as np
import ml_dtypes

BF = ml_dtypes.bfloat16

D1, D2, D3 = 32, 128, 128
NTOT = D1 * D2 * D3
FREE = D1 * D3  # 4096
B = 16
NCORES = 8

# single 128-wide const slots
S_F2R, S_F2I, S_F2In, S_BDR, S_BDI, S_BDIn, S_F2Rs, S_F2Is = range(8)
NSNG = 8


def _consts_np():
    k = np.arange(128)
    F2 = np.exp(-2j * np.pi * np.outer(k, k) / 128)
    k1 = np.arange(32)
    F1 = np.exp(-2j * np.pi * np.outer(k1, k1) / 32)
    BD = np.zeros((128, 128), complex)
    for g in range(4):
        BD[32 * g:32 * g + 32, 32 * g:32 * g + 32] = F1
    alpha = 1.0 / (NTOT * np.sqrt(np.float64(NTOT)))
    F2R, F2I = F2.real, F2.imag
    BDR, BDI = BD.real, BD.imag
    singles = [F2R, F2I, -F2I, BDR, BDI, -BDI, F2R * alpha, F2I * alpha]
    mats = np.concatenate(singles, axis=1)
    return np.ascontiguousarray(mats, dtype=np.float32).astype(BF)


def _build_program():
    import concourse.mybir as mybir
    import concourse.tile as tile
    from concourse import bacc

    f32 = mybir.dt.float32
    bf16 = mybir.dt.bfloat16

    nc = bacc.Bacc("TRN2")
    x0_d = nc.dram_tensor("x0", (D2, D1, D3), bf16, kind="ExternalInput")
    x1_d = nc.dram_tensor("x1", (D2, D1, D3), bf16, kind="ExternalInput")
    w_d = nc.dram_tensor("w", (D2, D1, D3), bf16, kind="ExternalInput")
    CW = NSNG * 128
    c_d = nc.dram_tensor("consts", (128, CW), bf16, kind="ExternalInput")
    y0_d = nc.dram_tensor("y0", (D2, D1, D3), f32, kind="ExternalOutput")
    y1_d = nc.dram_tensor("y1", (D2, D1, D3), f32, kind="ExternalOutput")

    with tile.TileContext(nc) as tc:
        with (
            tc.tile_pool(name="sb", bufs=1) as sb,
            tc.tile_pool(name="tp", bufs=2) as tp,
            tc.tile_pool(name="ps", bufs=2, space="PSUM") as ps,
        ):
            consts = sb.tile([128, CW], bf16, name="consts")
            nc.sync.dma_start(out=consts, in_=c_d.ap())

            def M1(i):
                return consts[:, 128 * i:128 * (i + 1)]

            def vol(name, n=2, dt=bf16, cols=FREE):
                return [sb.tile([128, cols], dt, name=f"{name}{c}")
                        for c in range(n)]

            zA = vol("zA")
            zB = vol("zB")
            VV = vol("VV")
            wA = vol("wA", 1)
            wB = vol("wB")
            wC = vol("wC")
            WT = vol("WT")
            yst = vol("yst", 2, f32)

            nc.sync.dma_start(
                out=wA[0].rearrange("p (a c) -> p a c", a=D1),
                in_=w_d.ap())
            for t in range(2):
                for comp, src in ((0, x0_d), (1, x1_d)):
                    nc.sync.dma_start(
                        out=zA[comp][:, 2048 * t:2048 * (t + 1)].rearrange(
                            "p (a c) -> p a c", a=16),
                        in_=src.ap()[:, 16 * t:16 * (t + 1), :])

            ectr = [0]

            def evict(dst, src):
                # pair-split across engines so the PSUM slot frees after ~one
                # copy latency; alternate the leader for balance
                lead_v = (ectr[0] // 2) % 2 == 0
                use_v = (ectr[0] % 2 == 0) == lead_v
                if use_v:
                    nc.vector.tensor_copy(dst, src)
                else:
                    nc.scalar.copy(dst, src)
                ectr[0] += 1

            def lhs_for(src, b, stat):
                if stat == "contig":
                    return src[:, 128 * b:128 * (b + 1)]
                if stat == "s2":
                    # f = 1024*k2q + 32*d1 + j; block j: m=(k2q,d1) stride 32
                    v = src.rearrange("p (m j) -> p m j", m=128, j=32)
                    return v[:, :, b:b + 1]
                # "stride32": f = 32*k2 + d1; block d1: k2 at stride 32
                v = src.rearrange("p (k2 d1) -> p k2 d1", k2=128, d1=32)
                return v[:, :, b:b + 1]

            def fused_group(dsts, srcs, mats, g, stat="contig",
                            real_in=False, scatter=None):
                """one 8-block psum group of a fused (data-stationary) stage.
                psum = [R-half 1024 | I-half 1024];
                pR[q] = sR_b^T mA (+ sI_b^T mB); pI[q] = sR_b^T mC (+ sI_b^T mA)
                """
                mA, mB, mC = mats
                pt = ps.tile([128, 2048], f32, name="pt", tag="ps")
                for q in range(8):
                    b = 8 * g + q
                    oR = slice(128 * q, 128 * (q + 1))
                    oI = slice(1024 + 128 * q, 1024 + 128 * (q + 1))
                    st = (q % 4 == 0)
                    sp = (q % 4 == 3)
                    lR = lhs_for(srcs[0], b, stat)
                    if real_in:
                        nc.tensor.matmul(pt[:, oR], lR, M1(mA),
                                         start=st, stop=sp,
                                         skip_group_check=True)
                        nc.tensor.matmul(pt[:, oI], lR, M1(mC),
                                         start=st, stop=sp,
                                         skip_group_check=True)
                    else:
                        lI = lhs_for(srcs[1], b, stat)
                        nc.tensor.matmul(pt[:, oR], lR, M1(mA),
                                         start=st, stop=False,
                                         skip_group_check=True)
                        nc.tensor.matmul(pt[:, oI], lR, M1(mC),
                                         start=st, stop=False,
                                         skip_group_check=True)
                        nc.tensor.matmul(pt[:, oR], lI, M1(mB),
                                         start=False, stop=sp,
                                         skip_group_check=True)
                        nc.tensor.matmul(pt[:, oI], lI, M1(mA),
                                         start=False, stop=sp,
                                         skip_group_check=True)
                if scatter == "s1":
                    # dst f = 1024*k2q + 32*(8g+q) + j ; psum cols (q, j, k2q)
                    # enumerate (q, k2q, j): src strided reads, dst 32-contig
                    for comp in range(2):
                        sv = pt[:, 1024 * comp:1024 * (comp + 1)].rearrange(
                            "p (q j k2q) -> p q k2q j", q=8, j=32, k2q=4)
                        dv = dsts[comp].rearrange(
                            "p (k2q d1 j) -> p d1 k2q j", k2q=4, d1=32, j=32)
                        evict(dv[:, 8 * g:8 * (g + 1), :, :], sv)
                else:
                    sl = slice(1024 * g, 1024 * (g + 1))
                    evict(dsts[0][:, sl], pt[:, :1024])
                    evict(dsts[1][:, sl], pt[:, 1024:])

            def std_group(dsts, srcs, mats, t):
                mA, mB, mC = mats
                pt = ps.tile([128, 2048], f32, name="pt", tag="ps")
                for h in range(2):
                    s = slice(1024 * t + 512 * h, 1024 * t + 512 * (h + 1))
                    oR = slice(512 * h, 512 * (h + 1))
                    oI = slice(1024 + 512 * h, 1024 + 512 * (h + 1))
                    nc.tensor.matmul(pt[:, oR], M1(mA), srcs[0][:, s],
                                     start=True, stop=False)
                    nc.tensor.matmul(pt[:, oI], M1(mC), srcs[0][:, s],
                                     start=True, stop=False)
                    nc.tensor.matmul(pt[:, oR], M1(mB), srcs[1][:, s],
                                     start=False, stop=True)
                    nc.tensor.matmul(pt[:, oI], M1(mA), srcs[1][:, s],
                                     start=False, stop=True)
                sl = slice(1024 * t, 1024 * (t + 1))
                evict(dsts[0][:, sl], pt[:, :1024])
                evict(dsts[1][:, sl], pt[:, 1024:])

            FWD_F2 = (S_F2R, S_F2In, S_F2I)
            INV_F2 = (S_F2R, S_F2I, S_F2In)
            FWD_BD = (S_BDR, S_BDIn, S_BDI)
            INV_BD = (S_BDR, S_BDI, S_BDIn)
            W_F2s = (S_F2Rs, None, S_F2Is)

            def mult_q(qq):
                s = slice(1024 * qq, 1024 * (qq + 1))
                t1 = tp.tile([128, 1024], bf16, name="t1", tag="t1")
                t2 = tp.tile([128, 1024], bf16, name="t2", tag="t2")
                nc.vector.tensor_tensor(t1, zB[0][:, s], WT[0][:, s],
                                        op=mybir.AluOpType.mult)
                nc.vector.tensor_tensor(t2, zB[1][:, s], WT[1][:, s],
                                        op=mybir.AluOpType.mult)
                nc.vector.tensor_tensor(VV[0][:, s], t1, t2,
                                        op=mybir.AluOpType.subtract)
                t3 = tp.tile([128, 1024], bf16, name="t3", tag="t1")
                t4 = tp.tile([128, 1024], bf16, name="t4", tag="t2")
                nc.vector.tensor_tensor(t3, zB[0][:, s], WT[1][:, s],
                                        op=mybir.AluOpType.mult)
                nc.vector.tensor_tensor(t4, zB[1][:, s], WT[0][:, s],
                                        op=mybir.AluOpType.mult)
                nc.vector.tensor_tensor(VV[1][:, s], t3, t4,
                                        op=mybir.AluOpType.add)

            # ---- schedule: W chain one stage ahead, interleaved with z ----
            for g in range(4):
                fused_group(wB, wA, W_F2s, g, real_in=True,
                            scatter="s1")                        # S1w
            for g in range(4):
                fused_group(zB, zA, FWD_F2, g, scatter="s1")     # S1 g
                fused_group(wC, wB, FWD_F2, g, stat="s2")        # S2w g
            for g in range(4):
                std_group(WT, wC, FWD_BD, g)                     # S3w g
                fused_group(zA, zB, FWD_F2, g, stat="s2")        # S2 g
            # S3 chunk t -> M quarter t -> S4 group t pipeline
            std_group(zB, zA, FWD_BD, 0)
            mult_q(0)
            std_group(zB, zA, FWD_BD, 1)
            mult_q(1)
            fused_group(zA, VV, INV_BD, 0)                       # S4 g0
            std_group(zB, zA, FWD_BD, 2)
            mult_q(2)
            fused_group(zA, VV, INV_BD, 1)
            std_group(zB, zA, FWD_BD, 3)
            mult_q(3)
            fused_group(zA, VV, INV_BD, 2)
            fused_group(zA, VV, INV_BD, 3)
            for g in range(4):
                fused_group(zB, zA, INV_F2, g, stat="stride32")  # S5
            for g in range(4):
                std_group(yst, zB, INV_F2, g)                    # S6
                for comp, dst in ((0, y0_d), (1, y1_d)):
                    nc.sync.dma_start(
                        out=dst.ap()[:, 8 * g:8 * (g + 1), :],
                        in_=yst[comp][:, 1024 * g:1024 * (g + 1)].rearrange(
                            "p (a c) -> p a c", a=8))
    return nc


_CACHE = {}


def _get_program():
    if "nc" not in _CACHE:
        nc = _build_program()
        try:
            if not nc.is_finalized():
                nc.finalize()
        except AttributeError:
            nc.finalize()
        _CACHE["nc"] = nc
    return _CACHE["nc"]


def _run(x, w_real, **kw):
    from concourse.bass_utils import run_bass_kernel_spmd

    nc = _get_program()
    consts = _consts_np()
    xT = np.ascontiguousarray(
        np.asarray(x, dtype=np.float32).transpose(0, 2, 1, 3)).astype(BF)
    wT = np.ascontiguousarray(
        np.asarray(w_real, dtype=np.float32).transpose(1, 0, 2)).astype(BF)
    in_maps = []
    for c in range(NCORES):
        in_maps.append({
            "x0": xT[2 * c],
            "x1": xT[2 * c + 1],
            "w": wT,
            "consts": consts,
        })
    res = run_bass_kernel_spmd(nc, in_maps, core_ids=list(range(NCORES)), **kw)
    out = np.empty((B, D1, D2, D3), dtype=np.float32)
    for c in range(NCORES):
        out[2 * c] = res.results[c]["y0"].transpose(1, 0, 2)
        out[2 * c + 1] = res.results[c]["y1"].transpose(1, 0, 2)
    return out, res


def kernel(x: np.ndarray, w_real: np.ndarray) -> np.ndarray:
    return _run(x, w_real)[0]


def kernel_traced(x: np.ndarray, w_real: np.ndarray):
    return _run(x, w_real, trace=True)


# revision 24
# speedup vs baseline: 1.1645x; 1.1645x over previous
"""Bass/Trainium2 kernel for batched 3D FFT circular convolution.

Reference computes y = Re(IFFT3(FFT3(x) * FFT3(w))) with net scaling
circular_conv3d(x, w) / sqrt(N); x: (16, 32, 128, 128) f32, w: (32, 128, 128).

Strategy (data parallel over batch, 8 cores x 2 samples):
- Pack two real samples as one complex volume z = x0 + i*x1; y0 = Re, y1 = Im.
- All FFTs as DFT matmuls in bf16 (inputs pre-rounded to bf16 on host).
- Transposes are FUSED into the DFT matmuls: the DFT matrix is symmetric, so
  making the DATA the stationary operand computes data^T @ F = (F @ data)^T --
  the transform output lands transposed (next axis on partitions) for free.
  Full-array weight loads hide in the PE background buffer; per block each
  component needs 2 LDW + 4 matmuls of 128 cols.
- PSUM is [R-half | I-half] so every eviction is a contiguous copy; layout
  permutations ride in single-strided stationary APs of the next stage:
    S1 evicts to f = 1024*k2q + 32*d1 + j  (32-elem contiguous j-runs), so
    S2's stationary for block j is (m = 32*k2q + d1) at stride 32 -- the
    exact (k2q,d1) partition order the block-diagonal d1-DFT needs.
    S4 evicts contiguously to f = 32*k2 + d1, so S5's stationary for block
    d1 is k2 at stride 32.
- Host pre-transposes x,w to (d2,d1,d3) and un-transposes y so every DMA run
  is >= 1KB contiguous (otherwise DMA is descriptor-rate bound).
- W~ = FFT3(w)/(N*sqrt(N)) computed on-device per core, replicated (an
  AllGather-sharded variant measured worse: cross-core start skew ~20us).
  W-chain stage groups interleave one stage ahead of the z-chain to fill
  the z-chain's all-to-all stage-boundary bubbles with PE work.

Stage layouts (partition | free):
  L0   [d2 | d1,d3]                  f = d1*128 + d3
  S1   fused FFT d2  -> [d3 | 1024*k2q + 32*d1 + j]   (k2 = 4j+k2q)
  S2   fused FFT d3  -> [(k2q,d1) | j,k3]  f = j*128 + k3
  S3   BD FFT d1 (weight-stationary) -> [(k2q,k1) | j,k3]
  M    V = Z * W~   (DVE, bf16 2x mode)
  S4   fused BD IFFT d1 -> [k3 | 32*k2 + d1]
  S5   fused IFFT k3 -> [k2 | d1,d3]  f = d1*128 + d3
  S6   IFFT k2 (weight-stationary) -> [d2 | d1,d3] -> DMA out
"""

import numpy as np
import ml_dtypes

BF = ml_dtypes.bfloat16

D1, D2, D3 = 32, 128, 128
NTOT = D1 * D2 * D3
FREE = D1 * D3  # 4096
B = 16
NCORES = 8

# paired (128x256) moving-operand const slots [A | B]
PF_F2, PI_F2, PF_F2s, PR_F2, PR_BDq, INV_F2_I, INV_BD_I = range(7)
NPAIR = 7
# single 128-wide const slots (weight-stationary stages)
S_F2R, S_F2I, S_F2In, S_BDR, S_BDI, S_BDIn = range(6)
NSNG = 6


def _consts_np():
    k = np.arange(128)
    F2 = np.exp(-2j * np.pi * np.outer(k, k) / 128)
    k1 = np.arange(32)
    F1 = np.exp(-2j * np.pi * np.outer(k1, k1) / 32)
    BD = np.zeros((128, 128), complex)
    for g in range(4):
        BD[32 * g:32 * g + 32, 32 * g:32 * g + 32] = F1
    alpha = 1.0 / (NTOT * np.sqrt(np.float64(NTOT)))
    F2R, F2I = F2.real, F2.imag
    BDR, BDI = BD.real, BD.imag
    pairs = [
        np.concatenate([F2R, F2I], axis=1),             # PF_F2   (fwd, sR)
        np.concatenate([-F2I, F2R], axis=1),            # PI_F2   (fwd, sI)
        np.concatenate([F2R * alpha, F2I * alpha], 1),  # PF_F2s  (w, real)
        np.concatenate([F2R, -F2I], axis=1),            # PR_F2   (inv, sR)
        np.concatenate([BDR, -BDI], axis=1),            # PR_BDq  (inv, sR)
        np.concatenate([F2I, F2R], axis=1),             # INV_F2_I (inv, sI)
        np.concatenate([BDI, BDR], axis=1),             # INV_BD_I (inv, sI)
    ]
    singles = [F2R, F2I, -F2I, BDR, BDI, -BDI]
    mats = np.concatenate([np.concatenate(pairs, axis=1),
                           np.concatenate(singles, axis=1)], axis=1)
    return np.ascontiguousarray(mats, dtype=np.float32).astype(BF)


def _build_program():
    import concourse.mybir as mybir
    import concourse.tile as tile
    from concourse import bacc

    f32 = mybir.dt.float32
    bf16 = mybir.dt.bfloat16

    nc = bacc.Bacc("TRN2")
    x0_d = nc.dram_tensor("x0", (D2, D1, D3), bf16, kind="ExternalInput")
    x1_d = nc.dram_tensor("x1", (D2, D1, D3), bf16, kind="ExternalInput")
    w_d = nc.dram_tensor("w", (D2, D1, D3), bf16, kind="ExternalInput")
    CW = NPAIR * 256 + NSNG * 128
    c_d = nc.dram_tensor("consts", (128, CW), bf16, kind="ExternalInput")
    y0_d = nc.dram_tensor("y0", (D2, D1, D3), f32, kind="ExternalOutput")
    y1_d = nc.dram_tensor("y1", (D2, D1, D3), f32, kind="ExternalOutput")

    with tile.TileContext(nc) as tc:
        with (
            tc.tile_pool(name="sb", bufs=1) as sb,
            tc.tile_pool(name="tp", bufs=2) as tp,
            tc.tile_pool(name="ps", bufs=2, space="PSUM") as ps,
        ):
            consts = sb.tile([128, CW], bf16, name="consts")
            nc.sync.dma_start(out=consts, in_=c_d.ap())

            def P2(i):
                return consts[:, 256 * i:256 * (i + 1)]

            def M1(i):
                o = NPAIR * 256
                return consts[:, o + 128 * i:o + 128 * (i + 1)]

            def vol(name, n=2, dt=bf16, cols=FREE):
                return [sb.tile([128, cols], dt, name=f"{name}{c}")
                        for c in range(n)]

            zA = vol("zA")
            zB = vol("zB")
            VV = vol("VV")
            wA = vol("wA", 1)
            wB = vol("wB")
            wC = vol("wC")
            WT = vol("WT")
            yst = vol("yst", 2, f32)

            nc.sync.dma_start(
                out=wA[0].rearrange("p (a c) -> p a c", a=D1),
                in_=w_d.ap())
            for t in range(2):
                for comp, src in ((0, x0_d), (1, x1_d)):
                    nc.sync.dma_start(
                        out=zA[comp][:, 2048 * t:2048 * (t + 1)].rearrange(
                            "p (a c) -> p a c", a=16),
                        in_=src.ap()[:, 16 * t:16 * (t + 1), :])

            ectr = [0]

            def evict(dst, src):
                # pair-split across engines so the PSUM slot frees after ~one
                # copy latency; alternate the leader for balance
                lead_v = (ectr[0] // 2) % 2 == 0
                use_v = (ectr[0] % 2 == 0) == lead_v
                if use_v:
                    nc.vector.tensor_copy(dst, src)
                else:
                    nc.scalar.copy(dst, src)
                ectr[0] += 1

            def lhs_for(src, b, stat):
                if stat == "contig":
                    return src[:, 128 * b:128 * (b + 1)]
                if stat == "s2":
                    # f = 1024*k2q + 32*d1 + j; block j: m=(k2q,d1) stride 32
                    v = src.rearrange("p (m j) -> p m j", m=128, j=32)
                    return v[:, :, b:b + 1]
                # "stride32": f = 32*k2 + d1; block d1: k2 at stride 32
                v = src.rearrange("p (k2 d1) -> p k2 d1", k2=128, d1=32)
                return v[:, :, b:b + 1]

            def fused_group(dsts, srcs, pairR, pairI, g, stat="contig",
                            real_in=False, scatter=None):
                """one 8-block psum group of a fused (data-stationary) stage;
                per block one 256-wide paired matmul per component:
                  psum[q] = [R|I] = sR_b^T @ P2(pairR) (+ sI_b^T @ P2(pairI))
                """
                pt = ps.tile([128, 2048], f32, name="pt", tag="ps")
                for q in range(8):
                    b = 8 * g + q
                    o = slice(256 * q, 256 * (q + 1))
                    st = (q % 2 == 0)
                    sp = (q % 2 == 1)
                    lR = lhs_for(srcs[0], b, stat)
                    if real_in:
                        nc.tensor.matmul(pt[:, o], lR, P2(pairR),
                                         start=st, stop=sp,
                                         skip_group_check=True)
                    else:
                        lI = lhs_for(srcs[1], b, stat)
                        nc.tensor.matmul(pt[:, o], lR, P2(pairR),
                                         start=st, stop=False,
                                         skip_group_check=True)
                        nc.tensor.matmul(pt[:, o], lI, P2(pairI),
                                         start=False, stop=sp,
                                         skip_group_check=True)
                if scatter == "s1":
                    # dst f = 1024*k2q + 32*(8g+q) + j ; psum cols (q,c,j,k2q)
                    # enumerate (q, k2q, j): strided src reads, 32-contig dst
                    sv = pt.rearrange("p (q c j k2q) -> p c q k2q j",
                                      q=8, c=2, j=32, k2q=4)
                    for comp in range(2):
                        dv = dsts[comp].rearrange(
                            "p (k2q d1 j) -> p d1 k2q j", k2q=4, d1=32, j=32)
                        evict(dv[:, 8 * g:8 * (g + 1), :, :], sv[:, comp])
                else:
                    # de-interleave [R|I] 128-col runs
                    pv = pt.rearrange("p (q c f) -> p c q f", q=8, c=2)
                    sl = slice(1024 * g, 1024 * (g + 1))
                    dv0 = dsts[0][:, sl].rearrange("p (q f) -> p q f", q=8)
                    dv1 = dsts[1][:, sl].rearrange("p (q f) -> p q f", q=8)
                    evict(dv0, pv[:, 0])
                    evict(dv1, pv[:, 1])

            def std_group(dsts, srcs, mats, t):
                mA, mB, mC = mats
                pt = ps.tile([128, 2048], f32, name="pt", tag="ps")
                for h in range(2):
                    s = slice(1024 * t + 512 * h, 1024 * t + 512 * (h + 1))
                    oR = slice(512 * h, 512 * (h + 1))
                    oI = slice(1024 + 512 * h, 1024 + 512 * (h + 1))
                    nc.tensor.matmul(pt[:, oR], M1(mA), srcs[0][:, s],
                                     start=True, stop=False)
                    nc.tensor.matmul(pt[:, oI], M1(mC), srcs[0][:, s],
                                     start=True, stop=False)
                    nc.tensor.matmul(pt[:, oR], M1(mB), srcs[1][:, s],
                                     start=False, stop=True)
                    nc.tensor.matmul(pt[:, oI], M1(mA), srcs[1][:, s],
                                     start=False, stop=True)
                sl = slice(1024 * t, 1024 * (t + 1))
                evict(dsts[0][:, sl], pt[:, :1024])
                evict(dsts[1][:, sl], pt[:, 1024:])

            FWD_BD = (S_BDR, S_BDIn, S_BDI)
            INV_F2 = (S_F2R, S_F2I, S_F2In)

            def mult_q(qq):
                s = slice(1024 * qq, 1024 * (qq + 1))
                t1 = tp.tile([128, 1024], bf16, name="t1", tag="t1")
                t2 = tp.tile([128, 1024], bf16, name="t2", tag="t2")
                nc.vector.tensor_tensor(t1, zB[0][:, s], WT[0][:, s],
                                        op=mybir.AluOpType.mult)
                nc.vector.tensor_tensor(t2, zB[1][:, s], WT[1][:, s],
                                        op=mybir.AluOpType.mult)
                nc.vector.tensor_tensor(VV[0][:, s], t1, t2,
                                        op=mybir.AluOpType.subtract)
                t3 = tp.tile([128, 1024], bf16, name="t3", tag="t1")
                t4 = tp.tile([128, 1024], bf16, name="t4", tag="t2")
                nc.vector.tensor_tensor(t3, zB[0][:, s], WT[1][:, s],
                                        op=mybir.AluOpType.mult)
                nc.vector.tensor_tensor(t4, zB[1][:, s], WT[0][:, s],
                                        op=mybir.AluOpType.mult)
                nc.vector.tensor_tensor(VV[1][:, s], t3, t4,
                                        op=mybir.AluOpType.add)

            # ---- schedule: W chain one stage ahead, interleaved with z ----
            for g in range(4):
                fused_group(wB, wA, PF_F2s, None, g, real_in=True,
                            scatter="s1")                        # S1w
            for g in range(4):
                fused_group(zB, zA, PF_F2, PI_F2, g, scatter="s1")  # S1 g
                fused_group(wC, wB, PF_F2, PI_F2, g, stat="s2")  # S2w g
            for g in range(4):
                std_group(WT, wC, FWD_BD, g)                     # S3w g
                fused_group(zA, zB, PF_F2, PI_F2, g, stat="s2")  # S2 g
            # S3 chunk t -> M quarter t -> S4 group t pipeline
            std_group(zB, zA, FWD_BD, 0)
            mult_q(0)
            std_group(zB, zA, FWD_BD, 1)
            mult_q(1)
            fused_group(zA, VV, PR_BDq, INV_BD_I, 0)             # S4 g0
            std_group(zB, zA, FWD_BD, 2)
            mult_q(2)
            fused_group(zA, VV, PR_BDq, INV_BD_I, 1)
            std_group(zB, zA, FWD_BD, 3)
            mult_q(3)
            fused_group(zA, VV, PR_BDq, INV_BD_I, 2)
            fused_group(zA, VV, PR_BDq, INV_BD_I, 3)
            for g in range(4):
                fused_group(zB, zA, PR_F2, INV_F2_I, g,
                            stat="stride32")                     # S5
            for g in range(4):
                std_group(yst, zB, INV_F2, g)                    # S6
                for comp, dst in ((0, y0_d), (1, y1_d)):
                    nc.sync.dma_start(
                        out=dst.ap()[:, 8 * g:8 * (g + 1), :],
                        in_=yst[comp][:, 1024 * g:1024 * (g + 1)].rearrange(
                            "p (a c) -> p a c", a=8))
    return nc


_CACHE = {}


def _get_program():
    if "nc" not in _CACHE:
        nc = _build_program()
        try:
            if not nc.is_finalized():
                nc.finalize()
        except AttributeError:
            nc.finalize()
        _CACHE["nc"] = nc
    return _CACHE["nc"]


def _run(x, w_real, **kw):
    from concourse.bass_utils import run_bass_kernel_spmd

    nc = _get_program()
    consts = _consts_np()
    xT = np.ascontiguousarray(
        np.asarray(x, dtype=np.float32).transpose(0, 2, 1, 3)).astype(BF)
    wT = np.ascontiguousarray(
        np.asarray(w_real, dtype=np.float32).transpose(1, 0, 2)).astype(BF)
    in_maps = []
    for c in range(NCORES):
        in_maps.append({
            "x0": xT[2 * c],
            "x1": xT[2 * c + 1],
            "w": wT,
            "consts": consts,
        })
    res = run_bass_kernel_spmd(nc, in_maps, core_ids=list(range(NCORES)), **kw)
    out = np.empty((B, D1, D2, D3), dtype=np.float32)
    for c in range(NCORES):
        out[2 * c] = res.results[c]["y0"].transpose(1, 0, 2)
        out[2 * c + 1] = res.results[c]["y1"].transpose(1, 0, 2)
    return out, res


def kernel(x: np.ndarray, w_real: np.ndarray) -> np.ndarray:
    return _run(x, w_real)[0]


def kernel_traced(x: np.ndarray, w_real: np.ndarray):
    return _run(x, w_real, trace=True)


# revision 26
# speedup vs baseline: 1.3843x; 1.1887x over previous
"""Bass/Trainium2 kernel for batched 3D FFT circular convolution.

Reference computes y = Re(IFFT3(FFT3(x) * FFT3(w))) with net scaling
circular_conv3d(x, w) / sqrt(N); x: (16, 32, 128, 128) f32, w: (32, 128, 128).

Strategy (data parallel over batch, 8 cores x 2 samples):
- Pack two real samples as one complex volume z = x0 + i*x1; y0 = Re, y1 = Im.
- All FFTs as DFT matmuls in bf16 (inputs pre-rounded to bf16 on host).
- Transposes are FUSED into the DFT matmuls: the DFT matrix is symmetric, so
  making the DATA the stationary operand computes data^T @ F = (F @ data)^T --
  the transform output lands transposed (next axis on partitions) for free.
  Full-array weight loads hide in the PE background buffer; per block each
  component needs 2 LDW + 4 matmuls of 128 cols.
- PSUM is [R-half | I-half] so every eviction is a contiguous copy; layout
  permutations ride in single-strided stationary APs of the next stage:
    S1 evicts to f = 1024*k2q + 32*d1 + j  (32-elem contiguous j-runs), so
    S2's stationary for block j is (m = 32*k2q + d1) at stride 32 -- the
    exact (k2q,d1) partition order the block-diagonal d1-DFT needs.
    S4 evicts contiguously to f = 32*k2 + d1, so S5's stationary for block
    d1 is k2 at stride 32.
- Host pre-transposes x,w to (d2,d1,d3) and un-transposes y so every DMA run
  is >= 1KB contiguous (otherwise DMA is descriptor-rate bound).
- W~ = FFT3(w)/(N*sqrt(N)) computed on-device per core, replicated (an
  AllGather-sharded variant measured worse: cross-core start skew ~20us).
  W-chain stage groups interleave one stage ahead of the z-chain to fill
  the z-chain's all-to-all stage-boundary bubbles with PE work.

Stage layouts (partition | free):
  L0   [d2 | d1,d3]                  f = d1*128 + d3
  S1   fused FFT d2  -> [d3 | 1024*k2q + 32*d1 + j]   (k2 = 4j+k2q)
  S2   fused FFT d3  -> [(k2q,d1) | j,k3]  f = j*128 + k3
  S3   BD FFT d1 (weight-stationary) -> [(k2q,k1) | j,k3]
  M    V = Z * W~   (DVE, bf16 2x mode)
  S4   fused BD IFFT d1 -> [k3 | 32*k2 + d1]
  S5   fused IFFT k3 -> [k2 | d1,d3]  f = d1*128 + d3
  S6   IFFT k2 (weight-stationary) -> [d2 | d1,d3] -> DMA out
"""

import numpy as np
import ml_dtypes

BF = ml_dtypes.bfloat16

D1, D2, D3 = 32, 128, 128
NTOT = D1 * D2 * D3
FREE = D1 * D3  # 4096
B = 16
NCORES = 8

# paired (128x256) moving-operand const slots [A | B]
PF_F2, PI_F2, PF_F2s, PR_F2, PR_BDq, INV_F2_I, INV_BD_I = range(7)
NPAIR = 7
# single 128-wide const slots (weight-stationary stages)
S_F2R, S_F2I, S_F2In, S_BDR, S_BDI, S_BDIn = range(6)
NSNG = 6


def _consts_np():
    k = np.arange(128)
    F2 = np.exp(-2j * np.pi * np.outer(k, k) / 128)
    k1 = np.arange(32)
    F1 = np.exp(-2j * np.pi * np.outer(k1, k1) / 32)
    BD = np.zeros((128, 128), complex)
    for g in range(4):
        BD[32 * g:32 * g + 32, 32 * g:32 * g + 32] = F1
    alpha = 1.0 / (NTOT * np.sqrt(np.float64(NTOT)))
    F2R, F2I = F2.real, F2.imag
    BDR, BDI = BD.real, BD.imag
    pairs = [
        np.concatenate([F2R, F2I], axis=1),             # PF_F2   (fwd, sR)
        np.concatenate([-F2I, F2R], axis=1),            # PI_F2   (fwd, sI)
        np.concatenate([F2R * alpha, F2I * alpha], 1),  # PF_F2s  (w, real)
        np.concatenate([F2R, -F2I], axis=1),            # PR_F2   (inv, sR)
        np.concatenate([BDR, -BDI], axis=1),            # PR_BDq  (inv, sR)
        np.concatenate([F2I, F2R], axis=1),             # INV_F2_I (inv, sI)
        np.concatenate([BDI, BDR], axis=1),             # INV_BD_I (inv, sI)
    ]
    singles = [F2R, F2I, -F2I, BDR, BDI, -BDI]
    mats = np.concatenate([np.concatenate(pairs, axis=1),
                           np.concatenate(singles, axis=1)], axis=1)
    return np.ascontiguousarray(mats, dtype=np.float32).astype(BF)


def _build_program():
    import concourse.mybir as mybir
    import concourse.tile as tile
    from concourse import bacc

    f32 = mybir.dt.float32
    bf16 = mybir.dt.bfloat16

    nc = bacc.Bacc("TRN2")
    x0_d = nc.dram_tensor("x0", (D2, D1, D3), bf16, kind="ExternalInput")
    x1_d = nc.dram_tensor("x1", (D2, D1, D3), bf16, kind="ExternalInput")
    w_d = nc.dram_tensor("w", (D2, D1, D3), bf16, kind="ExternalInput")
    CW = NPAIR * 256 + NSNG * 128
    c_d = nc.dram_tensor("consts", (128, CW), bf16, kind="ExternalInput")
    y0_d = nc.dram_tensor("y0", (D2, D1, D3), f32, kind="ExternalOutput")
    y1_d = nc.dram_tensor("y1", (D2, D1, D3), f32, kind="ExternalOutput")

    with tile.TileContext(nc) as tc:
        with (
            tc.tile_pool(name="sb", bufs=1) as sb,
            tc.tile_pool(name="tp", bufs=2) as tp,
            tc.tile_pool(name="ps", bufs=4, space="PSUM") as ps,
        ):
            consts = sb.tile([128, CW], bf16, name="consts")
            nc.sync.dma_start(out=consts, in_=c_d.ap())

            def P2(i):
                return consts[:, 256 * i:256 * (i + 1)]

            def M1(i):
                o = NPAIR * 256
                return consts[:, o + 128 * i:o + 128 * (i + 1)]

            def vol(name, n=2, dt=bf16, cols=FREE):
                return [sb.tile([128, cols], dt, name=f"{name}{c}")
                        for c in range(n)]

            zA = vol("zA")
            zB = vol("zB")
            VV = vol("VV")
            wA = vol("wA", 1)
            wB = vol("wB")
            wC = vol("wC")
            WT = vol("WT")
            yst = vol("yst", 2, f32)

            nc.sync.dma_start(
                out=wA[0].rearrange("p (a c) -> p a c", a=D1),
                in_=w_d.ap())
            for t in range(2):
                for comp, src in ((0, x0_d), (1, x1_d)):
                    nc.sync.dma_start(
                        out=zA[comp][:, 2048 * t:2048 * (t + 1)].rearrange(
                            "p (a c) -> p a c", a=16),
                        in_=src.ap()[:, 16 * t:16 * (t + 1), :])

            ectr = [0]

            def evict(dst, src, pref=None):
                # pair-split across engines so the PSUM slot frees after ~one
                # copy latency; alternate the leader for balance
                if pref == "s":
                    nc.scalar.copy(dst, src)
                    return
                lead_v = (ectr[0] // 2) % 2 == 0
                use_v = (ectr[0] % 2 == 0) == lead_v
                if use_v:
                    nc.vector.tensor_copy(dst, src)
                else:
                    nc.scalar.copy(dst, src)
                ectr[0] += 1

            def lhs_for(src, b, stat):
                if stat == "contig":
                    return src[:, 128 * b:128 * (b + 1)]
                if stat == "s2":
                    # f = 1024*k2q + 32*d1 + j; block j: m=(k2q,d1) stride 32
                    v = src.rearrange("p (m j) -> p m j", m=128, j=32)
                    return v[:, :, b:b + 1]
                # "stride32": f = 32*k2 + d1; block d1: k2 at stride 32
                v = src.rearrange("p (k2 d1) -> p k2 d1", k2=128, d1=32)
                return v[:, :, b:b + 1]

            def fused_group(dsts, srcs, pairR, pairI, g, stat="contig",
                            real_in=False, scatter=None, pref=None):
                """one 8-block psum group of a fused (data-stationary) stage;
                per block one 256-wide paired matmul per component:
                  psum[q] = [R|I] = sR_b^T @ P2(pairR) (+ sI_b^T @ P2(pairI))
                """
                pt = ps.tile([128, 1024], f32, name="pt", tag="ps")
                for q in range(4):
                    b = 4 * g + q
                    o = slice(256 * q, 256 * (q + 1))
                    st = (q % 2 == 0)
                    sp = (q % 2 == 1)
                    lR = lhs_for(srcs[0], b, stat)
                    if real_in:
                        nc.tensor.matmul(pt[:, o], lR, P2(pairR),
                                         start=st, stop=sp,
                                         skip_group_check=True)
                    else:
                        lI = lhs_for(srcs[1], b, stat)
                        nc.tensor.matmul(pt[:, o], lR, P2(pairR),
                                         start=st, stop=False,
                                         skip_group_check=True)
                        nc.tensor.matmul(pt[:, o], lI, P2(pairI),
                                         start=False, stop=sp,
                                         skip_group_check=True)
                if scatter == "s1":
                    # dst f = 1024*k2q + 32*(8g+q) + j ; psum cols (q,c,j,k2q)
                    # enumerate (q, k2q, j): strided src reads, 32-contig dst
                    sv = pt.rearrange("p (q c j k2q) -> p c q k2q j",
                                      q=4, c=2, j=32, k2q=4)
                    for comp in range(2):
                        dv = dsts[comp].rearrange(
                            "p (k2q d1 j) -> p d1 k2q j", k2q=4, d1=32, j=32)
                        evict(dv[:, 4 * g:4 * (g + 1), :, :], sv[:, comp])
                else:
                    # de-interleave [R|I] 128-col runs
                    pv = pt.rearrange("p (q c f) -> p c q f", q=4, c=2)
                    sl = slice(512 * g, 512 * (g + 1))
                    dv0 = dsts[0][:, sl].rearrange("p (q f) -> p q f", q=4)
                    dv1 = dsts[1][:, sl].rearrange("p (q f) -> p q f", q=4)
                    evict(dv0, pv[:, 0], pref=pref)
                    evict(dv1, pv[:, 1], pref=pref)

            def std_group(dsts, srcs, mats, t, pref=None):
                mA, mB, mC = mats
                pt = ps.tile([128, 1024], f32, name="pt", tag="ps")
                s = slice(512 * t, 512 * (t + 1))
                nc.tensor.matmul(pt[:, :512], M1(mA), srcs[0][:, s],
                                 start=True, stop=False)
                nc.tensor.matmul(pt[:, 512:], M1(mC), srcs[0][:, s],
                                 start=True, stop=False)
                nc.tensor.matmul(pt[:, :512], M1(mB), srcs[1][:, s],
                                 start=False, stop=True)
                nc.tensor.matmul(pt[:, 512:], M1(mA), srcs[1][:, s],
                                 start=False, stop=True)
                evict(dsts[0][:, s], pt[:, :512], pref=pref)
                evict(dsts[1][:, s], pt[:, 512:], pref=pref)

            FWD_BD = (S_BDR, S_BDIn, S_BDI)
            INV_F2 = (S_F2R, S_F2I, S_F2In)

            def mult_q(qq):
                s = slice(1024 * qq, 1024 * (qq + 1))
                t1 = tp.tile([128, 1024], bf16, name="t1", tag="t1")
                t2 = tp.tile([128, 1024], bf16, name="t2", tag="t2")
                nc.vector.tensor_tensor(t1, zB[0][:, s], WT[0][:, s],
                                        op=mybir.AluOpType.mult)
                nc.vector.tensor_tensor(t2, zB[1][:, s], WT[1][:, s],
                                        op=mybir.AluOpType.mult)
                nc.vector.tensor_tensor(VV[0][:, s], t1, t2,
                                        op=mybir.AluOpType.subtract)
                t3 = tp.tile([128, 1024], bf16, name="t3", tag="t1")
                t4 = tp.tile([128, 1024], bf16, name="t4", tag="t2")
                nc.vector.tensor_tensor(t3, zB[0][:, s], WT[1][:, s],
                                        op=mybir.AluOpType.mult)
                nc.vector.tensor_tensor(t4, zB[1][:, s], WT[0][:, s],
                                        op=mybir.AluOpType.mult)
                nc.vector.tensor_tensor(VV[1][:, s], t3, t4,
                                        op=mybir.AluOpType.add)

            # ---- schedule: W chain one stage ahead, interleaved with z ----
            for g in range(8):
                fused_group(wB, wA, PF_F2s, None, g, real_in=True,
                            scatter="s1")                        # S1w
            for g in range(8):
                fused_group(zB, zA, PF_F2, PI_F2, g, scatter="s1")  # S1 g
                fused_group(wC, wB, PF_F2, PI_F2, g, stat="s2")  # S2w g
            for g in range(8):
                std_group(WT, wC, FWD_BD, g)                     # S3w g
                fused_group(zA, zB, PF_F2, PI_F2, g, stat="s2")  # S2 g
            # S3 chunks -> M quarter -> S4 groups pipeline; region evictions
            # on ACT so DVE stays free for the multiply
            std_group(zB, zA, FWD_BD, 0, pref="s")
            std_group(zB, zA, FWD_BD, 1, pref="s")
            mult_q(0)
            std_group(zB, zA, FWD_BD, 2, pref="s")
            std_group(zB, zA, FWD_BD, 3, pref="s")
            mult_q(1)
            fused_group(zA, VV, PR_BDq, INV_BD_I, 0, pref="s")   # S4 g0
            fused_group(zA, VV, PR_BDq, INV_BD_I, 1, pref="s")
            std_group(zB, zA, FWD_BD, 4, pref="s")
            std_group(zB, zA, FWD_BD, 5, pref="s")
            mult_q(2)
            fused_group(zA, VV, PR_BDq, INV_BD_I, 2, pref="s")
            fused_group(zA, VV, PR_BDq, INV_BD_I, 3, pref="s")
            std_group(zB, zA, FWD_BD, 6, pref="s")
            std_group(zB, zA, FWD_BD, 7, pref="s")
            mult_q(3)
            for g in range(4, 8):
                fused_group(zA, VV, PR_BDq, INV_BD_I, g)         # S4 g4-7
            for g in range(8):
                fused_group(zB, zA, PR_F2, INV_F2_I, g,
                            stat="stride32")                     # S5
            for g in range(8):
                std_group(yst, zB, INV_F2, g)                    # S6
            for g in range(4):
                for comp, dst in ((0, y0_d), (1, y1_d)):
                    nc.sync.dma_start(
                        out=dst.ap()[:, 8 * g:8 * (g + 1), :],
                        in_=yst[comp][:, 1024 * g:1024 * (g + 1)].rearrange(
                            "p (a c) -> p a c", a=8))
    return nc


_CACHE = {}


def _get_program():
    if "nc" not in _CACHE:
        nc = _build_program()
        try:
            if not nc.is_finalized():
                nc.finalize()
        except AttributeError:
            nc.finalize()
        _CACHE["nc"] = nc
    return _CACHE["nc"]


def _run(x, w_real, **kw):
    from concourse.bass_utils import run_bass_kernel_spmd

    nc = _get_program()
    consts = _consts_np()
    xT = np.ascontiguousarray(
        np.asarray(x, dtype=np.float32).transpose(0, 2, 1, 3)).astype(BF)
    wT = np.ascontiguousarray(
        np.asarray(w_real, dtype=np.float32).transpose(1, 0, 2)).astype(BF)
    in_maps = []
    for c in range(NCORES):
        in_maps.append({
            "x0": xT[2 * c],
            "x1": xT[2 * c + 1],
            "w": wT,
            "consts": consts,
        })
    res = run_bass_kernel_spmd(nc, in_maps, core_ids=list(range(NCORES)), **kw)
    out = np.empty((B, D1, D2, D3), dtype=np.float32)
    for c in range(NCORES):
        out[2 * c] = res.results[c]["y0"].transpose(1, 0, 2)
        out[2 * c + 1] = res.results[c]["y1"].transpose(1, 0, 2)
    return out, res


def kernel(x: np.ndarray, w_real: np.ndarray) -> np.ndarray:
    return _run(x, w_real)[0]


def kernel_traced(x: np.ndarray, w_real: np.ndarray):
    return _run(x, w_real, trace=True)
